# revision 19
# baseline (speedup 1.0000x reference)
"""Trainium2 Bass kernel for nn_AttentionModule (sparse_attention).

Computation (reference):
  q = tanh(einsum('hde,be->hbd', Query, x))          H=8 D=256 E=1536
  k = tanh(einsum('hdf,blf->hbld', Key, bank))       B=64 L=256 F=768
  s = einsum('hbld,hbd->hbl', k, q)  masked softmax over l
  out = LeakyReLU_0.4(einsum('hbl,blf->bhf', attn, bank))

Strategy: data-parallel over batch B across 8 NeuronCores (8 b's per core).

Sparsity: the mask zeroes ~half the L positions; masked positions receive
-1e8 bias so their softmax weight is ~0 and they contribute nothing to the
output.  Host prep therefore COMPACTS bank per-b to the unmasked columns
(padded to Lp, a multiple of 32; padded slots keep the -1e8 bias), which
cuts the dominant k-matmul, the score matmul and the softmax by L/Lp
(~1.6x).  Compaction is a gather (re-layout); all FLOPs stay on device.

Device pipeline per core (PE stream is issued to stay gap-free):
  - k = tanh(KeyT^T @ bankT) head-outer (KeyT streams one head per ~5us of
    PE work), all 4 b-pairs; moving dim 2*Lp>=256 so fp32r runs full rate.
  - q-heads (bf16, halves the Query DMA) interleaved into the first k-heads
    to cover the DMA lead-in; q is DVE-block-transposed into the zero-padded
    score lhsT (qz) -- no PE transposes anywhere.
  - scores: all (h,dc) accumulate into one [40, 2*Lp] psum per b-pair;
    masked softmax on ACT/DVE; attn DVE-block-transposed; emb = attn @ bank
    with normalize+LeakyReLU fused into one Prelu activation.
"""

import os
import numpy as np
import concourse.bass as bass  # noqa: F401
import concourse.mybir as mybir
import concourse.tile as tile
from concourse import bacc, bass_utils

F32 = mybir.dt.float32
F32R = mybir.dt.float32r
FP16 = mybir.dt.float16
AF = mybir.ActivationFunctionType
AX = mybir.AxisListType

# dtype of the big k-matmul operands (KeyT / bankT). fp16 halves their DMA
# at ~0.05% quantization error; fp32r keeps tf32-grade accuracy.
KF16 = os.environ.get("KERNEL_KF16", "1") == "1"

H, D, E, F = 8, 256, 1536, 768
B, L = 64, 256
NCORES = 8
BPC = B // NCORES          # 8 b's per core
NBP = BPC // 2             # 4 b-pairs per core
EC, FC, DC = E // 128, F // 128, D // 128   # 12, 6, 2


def _build_program(Lp, kf16):
    KMM = FP16 if kf16 else F32R
    N2 = 2 * Lp                 # k / score moving width per b-pair
    Lpt = -(-Lp // 32) * 32     # Lp padded to the DVE 32x32 transpose grid
    NBLK = Lpt // 32            # transpose blocks per attn stack
    L_REM = Lp - 128 if Lp > 128 else 0   # l rows beyond the first 128

    nc = bacc.Bacc("TRN2", target_bir_lowering=False, debug=False,
                   enable_asserts=False, num_devices=NCORES)
    qt = nc.dram_tensor("qt", [H, E, D], FP16, kind="ExternalInput").ap()
    kt = nc.dram_tensor("kt", [H, F, D], KMM, kind="ExternalInput").ap()
    bkt = nc.dram_tensor("bkt", [NBP, F, N2], KMM, kind="ExternalInput").ap()
    bkn = nc.dram_tensor("bkn", [BPC, Lp, F], FP16, kind="ExternalInput").ap()
    xt = nc.dram_tensor("xt", [128, EC * BPC], FP16, kind="ExternalInput").ap()
    mb = nc.dram_tensor("mb", [BPC, H, Lp], F32, kind="ExternalInput").ap()
    out = nc.dram_tensor("out", [BPC, H, F], F32, kind="ExternalOutput").ap()

    with tile.TileContext(nc) as tc:
        with tc.tile_pool(name="const", bufs=1) as cpool, \
             tc.tile_pool(name="weights", bufs=1) as wpool, \
             tc.tile_pool(name="stream", bufs=4) as spool, \
             tc.tile_pool(name="small", bufs=2) as smpool, \
             tc.tile_pool(name="psK", bufs=3, space="PSUM") as psK, \
             tc.tile_pool(name="psS", bufs=2, space="PSUM") as psS, \
             tc.tile_pool(name="psM", bufs=3, space="PSUM") as psM:

            # ---------------- resident SBUF tiles ------------------------
            kt_tiles = [wpool.tile([128, FC * D], KMM, name=f"kt_sb{h}",
                                   tag=f"kt_sb{h}") for h in range(H)]
            # bankT, all b-pairs resident: [128(f), bp, fc, (b2 l)]
            bktA = cpool.tile([128, NBP * FC * N2], KMM)
            bktA_v = bktA[:].rearrange("p (bp fc n) -> p bp fc n", bp=NBP, fc=FC)
            # bank (natural layout), emb rhs: first 128 l-rows + remainder.
            bkn0 = cpool.tile([128, BPC * F], FP16, name="bkn0")
            bkn1 = (cpool.tile([L_REM, BPC * F], FP16, name="bkn1")
                    if L_REM else None)
            xt_sb = cpool.tile([128, EC * BPC], FP16)
            mbA = cpool.tile([8, BPC * Lp], F32)
            # zero-padded score lhsT: col = bp*640 + (2h+dc)*40 + 32*b2 + h
            qz = cpool.tile([128, NBP * 640], FP16)
            qz_v = qz[:].rearrange("p (bp blk c) -> p bp blk c", bp=NBP, blk=16)
            # k = tanh(...), all heads/pairs resident: [128(d), h, dc, bp, n2]
            k_t = cpool.tile([128, H * DC * NBP * N2], FP16)
            k_v = k_t[:].rearrange("p (h dc bp n) -> p h dc bp n",
                                   h=H, dc=DC, bp=NBP)

            # ---------------- DMA issue helpers --------------------------
            def load_xt_mb():
                nc.sync.dma_start(xt_sb[:], xt)
                nc.sync.dma_start(
                    mbA[:].rearrange("h (b l) -> h b l", b=BPC),
                    mb.rearrange("b h l -> h b l"))

            def load_kt(h, pieces=2):
                fc_per = FC // pieces
                for piece in range(pieces):
                    nc.sync.dma_start(
                        kt_tiles[h][:, piece * fc_per * D:(piece + 1) * fc_per * D]
                        .rearrange("p (fc d) -> p fc d", fc=fc_per),
                        kt[h, piece * fc_per * 128:(piece + 1) * fc_per * 128]
                        .rearrange("(fc p) d -> p fc d", p=128))

            def load_kt_piece(h, piece, pieces):
                fc_per = FC // pieces
                nc.sync.dma_start(
                    kt_tiles[h][:, piece * fc_per * D:(piece + 1) * fc_per * D]
                    .rearrange("p (fc d) -> p fc d", fc=fc_per),
                    kt[h, piece * fc_per * 128:(piece + 1) * fc_per * 128]
                    .rearrange("(fc p) d -> p fc d", p=128))

            def load_bktA_piece(bp, piece, pieces):
                fc_per = FC // pieces
                nc.sync.dma_start(
                    bktA_v[:, bp, piece * fc_per:(piece + 1) * fc_per],
                    bkt[bp, piece * fc_per * 128:(piece + 1) * fc_per * 128]
                    .rearrange("(fc p) n -> p fc n", p=128))

            def load_bktA(bp, pieces=2):
                fc_per = FC // pieces
                for piece in range(pieces):
                    nc.sync.dma_start(
                        bktA_v[:, bp, piece * fc_per:(piece + 1) * fc_per],
                        bkt[bp, piece * fc_per * 128:(piece + 1) * fc_per * 128]
                        .rearrange("(fc p) n -> p fc n", p=128))

            def load_bkn():
                for b in range(BPC):
                    nc.sync.dma_start(
                        bkn0[:, b * F:(b + 1) * F], bkn[b, 0:128])
                    if L_REM:
                        nc.sync.dma_start(
                            bkn1[:, b * F:(b + 1) * F], bkn[b, 128:128 + L_REM])

            # ---------------- q phase (bf16) ------------------------------
            # q = tanh(x @ Query^T); two stacks of 4 heads (rows 32*hh, 8
            # live rows each -- engine writes need 32-aligned partition
            # bases) -> DVE 32x32 block transposes -> strided copies into qz.
            q_stacks = [smpool.tile([128, D], FP16, name=f"qs{g}", tag=f"qs{g}")
                        for g in range(2)]

            def q_head(h):
                g, hh = divmod(h, 4)
                pq = psM.tile([BPC, D], F32, name="pq", tag="psm")
                qt_c = spool.tile([128, EC * D], FP16, name="qt_c", tag="qt_c")
                nc.sync.dma_start(
                    qt_c[:].rearrange("p (ec d) -> p ec d", ec=EC),
                    qt[h].rearrange("(ec p) d -> p ec d", p=128))
                for ec in range(EC):
                    nc.tensor.matmul(pq[:], xt_sb[:, ec * BPC:(ec + 1) * BPC],
                                     qt_c[:, ec * D:(ec + 1) * D],
                                     start=(ec == 0), stop=(ec == EC - 1))
                nc.scalar.activation(q_stacks[g][32 * hh:32 * hh + 8, :], pq[:],
                                     AF.Tanh)

            def q_scatter(g):
                for dc in range(DC):
                    qT = smpool.tile([128, 128], FP16, name=f"qT{g}{dc}",
                                     tag="qT")
                    for i in range(4):          # head row-blocks
                        for j in range(4):      # d sub-blocks
                            nc.vector.transpose(
                                qT[32 * j:32 * j + 32, 32 * i:32 * i + 32],
                                q_stacks[g][32 * i:32 * i + 32,
                                            dc * 128 + 32 * j:dc * 128 + 32 * j + 32])
                    # qT col = 32*hh + 2*bp + b2 (b = 2bp+b2 local batch)
                    qT_v = qT[:].rearrange("p (hh bpx b2) -> p hh bpx b2",
                                           hh=4, bpx=16)
                    for hh in range(4):
                        h = 4 * g + hh
                        for b2 in range(2):
                            nc.vector.tensor_copy(
                                qz_v[:, :, 2 * h + dc, 32 * b2 + h],
                                qT_v[:, hh, 0:4, b2])

            # ---------------- k phase -------------------------------------
            def k_head(h, bps=range(NBP)):
                for bp in bps:
                    for dc in range(DC):
                        pk = psK.tile([128, N2], F32, name="pk", tag="pk")
                        for fc in range(FC):
                            nc.tensor.matmul(
                                pk[:],
                                kt_tiles[h][:, fc * D + dc * 128:
                                            fc * D + dc * 128 + 128],
                                bktA_v[:, bp, fc],
                                start=(fc == 0), stop=(fc == FC - 1))
                        nc.scalar.activation(k_v[:, h, dc, bp], pk[:], AF.Tanh)

            # ---------------- score / softmax / emb -----------------------
            simsafe = os.environ.get("KERNEL_SIM_SAFE", "0") == "1"

            def score_mms(bp):
                ps40 = psS.tile([40, N2], F32, name="ps40", tag="ps40")
                for h in range(H):
                    for dc in range(DC):
                        nc.tensor.matmul(
                            ps40[:], qz_v[:, bp, 2 * h + dc],
                            k_v[:, h, dc, bp],
                            start=(h == 0 and dc == 0),
                            stop=(h == H - 1 and dc == DC - 1))
                return ps40

            def softmax(bp, ps40):
                # both b2 stacked at rows 0 / 32: one exp, one reciprocal,
                # one Prelu scale vector for the whole b-pair.
                s40 = smpool.tile([40, Lp], F32, name="s40", tag="s40")
                nm40 = smpool.tile([40, 1], F32, name="nm40", tag="nm40")
                zs40 = smpool.tile([40, 1], F32, name="zs40", tag="zs40")
                rz40 = smpool.tile([40, 1], F32, name="rz40", tag="rz40")
                p40 = smpool.tile([64, Lpt], FP16, name="p40", tag="p40")
                for b2 in range(2):
                    b = 2 * bp + b2
                    nc.vector.tensor_add(s40[32 * b2:32 * b2 + 8, :],
                                         ps40[32 * b2:32 * b2 + 8,
                                              Lp * b2:Lp * b2 + Lp],
                                         mbA[:, b * Lp:(b + 1) * Lp])
                    nc.vector.reduce_max(nm40[32 * b2:32 * b2 + 8, :],
                                         s40[32 * b2:32 * b2 + 8, :],
                                         axis=AX.X, negate=True)
                nc.scalar.activation(p40[0:40, 0:Lp], s40[:], AF.Exp,
                                     bias=nm40[:], accum_out=zs40[:])
                nc.vector.reciprocal(rz40[:], zs40[:])
                pT0 = smpool.tile([128, 64], FP16, name="pT0", tag="pT0")
                pT1 = (smpool.tile([max(L_REM, 32), 64], FP16,
                                   name="pT1", tag="pT1")
                       if L_REM else None)
                for i in range(2):              # b2 row-blocks
                    for j in range(NBLK):
                        row = 32 * j
                        dst = (pT0[row:row + 32, 32 * i:32 * i + 32]
                               if row < 128
                               else pT1[row - 128:row - 96, 32 * i:32 * i + 32])
                        nc.vector.transpose(
                            dst, p40[32 * i:32 * i + 32, row:row + 32])
                return rz40, pT0, pT1

            def emb(bp, rz40, pT0, pT1):
                o40 = smpool.tile([40, F], F32, name="o40", tag="o40")
                for fh in range(2):
                    pe = psM.tile([40, 384], F32, name="pe", tag="psm")
                    for b2 in range(2):
                        b = 2 * bp + b2
                        nc.tensor.matmul(
                            pe[32 * b2:32 * b2 + 8, :],
                            pT0[:, 32 * b2:32 * b2 + 8],
                            bkn0[:, b * F + fh * 384:b * F + fh * 384 + 384],
                            start=True, stop=(L_REM == 0))
                        if L_REM:
                            nc.tensor.matmul(
                                pe[32 * b2:32 * b2 + 8, :],
                                pT1[0:L_REM, 32 * b2:32 * b2 + 8],
                                bkn1[:, b * F + fh * 384:b * F + fh * 384 + 384],
                                start=False, stop=True)
                    if simsafe:
                        nc.scalar.activation(o40[:, fh * 384:fh * 384 + 384],
                                             pe[0:40, :], AF.Copy,
                                             scale=rz40[:])
                    else:
                        nc.scalar.activation(o40[:, fh * 384:fh * 384 + 384],
                                             pe[0:40, :], AF.Prelu,
                                             scale=rz40[:], alpha=0.4)
                for b2 in range(2):
                    nc.sync.dma_start(out[2 * bp + b2],
                                      o40[32 * b2:32 * b2 + 8, :])

            # ---------------- program order -------------------------------
            # Front-load the high-leverage DMAs: each kt[h] (1.1us of DMA)
            # unlocks 5.1us of PE work, bankT is shared by all heads.  The
            # Query stream (1:1 DMA:PE) queues behind them and is consumed
            # mid-sweep when everything is already resident.
            nc.vector.memset(qz[:], 0.0)
            for piece in range(6):
                load_kt_piece(0, piece, 6)
                load_bktA_piece(0, piece, 6)
            nc.sync.dma_start(xt_sb[:], xt)
            load_bktA(1, pieces=1)
            load_bktA(2, pieces=1)
            load_bktA(3, pieces=1)
            load_kt(1, pieces=1)
            load_kt(2, pieces=1)
            load_kt(3, pieces=1)
            k_head(0)
            k_head(1)
            k_head(2)
            for h in range(4):
                q_head(h)
            q_scatter(0)
            k_head(3)
            for h in range(4, H):
                q_head(h)
            q_scatter(1)
            load_kt(4, pieces=1)
            k_head(4)
            load_kt(5, pieces=1)
            nc.sync.dma_start(
                mbA[:].rearrange("h (b l) -> h b l", b=BPC),
                mb.rearrange("b h l -> h b l"))
            k_head(5)
            load_kt(6, pieces=1)
            load_bkn()
            k_head(6)
            load_kt(7, pieces=1)
            k_head(7)

            # all scores, then all embs: the last softmax chain overlaps
            # the first embs, so the PE never waits on it.
            ps0 = score_mms(0)
            sm0 = softmax(0, ps0)
            ps1 = score_mms(1)
            sm1 = softmax(1, ps1)
            ps2 = score_mms(2)
            sm2 = softmax(2, ps2)
            ps3 = score_mms(3)
            sm3 = softmax(3, ps3)
            emb(0, *sm0)
            emb(1, *sm1)
            emb(2, *sm2)
            emb(3, *sm3)

    nc.finalize()
    return nc


def _host_prep(x, bank, mask, Query, Key, Lp, kf16):
    x = np.ascontiguousarray(x, dtype=np.float32)
    bank = np.ascontiguousarray(bank, dtype=np.float32)
    Query = np.ascontiguousarray(Query, dtype=np.float32)
    Key = np.ascontiguousarray(Key, dtype=np.float32)
    mask = np.asarray(mask)

    kdt = np.float16 if kf16 else np.float32
    qt = np.ascontiguousarray(Query.transpose(0, 2, 1)).astype(np.float16)
    kt = np.ascontiguousarray(Key.transpose(0, 2, 1)).astype(kdt)  # [H, F, D]

    # per-b compaction of bank to its unmasked columns, padded to Lp
    if Lp == L:
        bank_c = bank
        mbias = np.where(mask == 0, np.float32(-1e8), np.float32(0.0))
    else:
        idx = np.zeros((B, Lp), dtype=np.int64)
        mbias = np.full((B, Lp), np.float32(-1e8))
        for b in range(B):
            nz = np.flatnonzero(mask[b])
            idx[b, :len(nz)] = nz
            mbias[b, :len(nz)] = 0.0
        bank_c = np.take_along_axis(bank, idx[:, :, None], axis=1)
    mbias = mbias.astype(np.float32)

    in_maps = []
    for c in range(NCORES):
        bs = c * BPC
        bc = bank_c[bs:bs + BPC]                      # [BPC, Lp, F]
        # bkt: [NBP, F, 2*Lp] -- b-pair minor so one DMA pair per bp
        bkt_c = np.ascontiguousarray(
            bc.reshape(NBP, 2, Lp, F).transpose(0, 3, 1, 2)
            .reshape(NBP, F, 2 * Lp)).astype(kdt)
        xs = x[bs:bs + BPC]                           # [BPC, E]
        xt_c = np.ascontiguousarray(
            xs.T.reshape(EC, 128, BPC).transpose(1, 0, 2)
            .reshape(128, EC * BPC)).astype(np.float16)
        mb_c = np.ascontiguousarray(
            np.repeat(mbias[bs:bs + BPC, None, :], H, axis=1))
        in_maps.append({
            "qt": qt,
            "kt": kt,
            "bkt": bkt_c,
            "bkn": np.ascontiguousarray(bc).astype(np.float16),
            "xt": xt_c,
            "mb": mb_c,
        })
    return in_maps


_NC_CACHE = {}


def _pick_lp(mask):
    counts = np.asarray(mask).astype(bool).sum(axis=1)
    if counts.min() == 0:
        return L
    return int(min(L, max(128, -(-int(counts.max()) // 8) * 8)))


def kernel(x, bank, mask, Query, Key):
    Lp = _pick_lp(mask)
    key = (Lp, KF16)
    if key not in _NC_CACHE:
        _NC_CACHE[key] = _build_program(Lp, KF16)
    nc = _NC_CACHE[key]
    in_maps = _host_prep(x, bank, mask, Query, Key, Lp, KF16)

    trace = os.environ.get("KERNEL_TRACE", "0") == "1"
    res = bass_utils.run_bass_kernel_spmd(nc, in_maps,
                                          core_ids=list(range(NCORES)),
                                          trace=trace)
    if trace:
        print("exec_time_ns:", res.exec_time_ns,
              "mean:", res.mean_exec_time_ns,
              "core:", res.max_exec_time_core_id)
    return np.concatenate([r["out"] for r in res.results], axis=0)


# revision 20
# speedup vs baseline: 1.0437x; 1.0437x over previous
"""Trainium2 Bass kernel for nn_AttentionModule (sparse_attention).

Computation (reference):
  q = tanh(einsum('hde,be->hbd', Query, x))          H=8 D=256 E=1536
  k = tanh(einsum('hdf,blf->hbld', Key, bank))       B=64 L=256 F=768
  s = einsum('hbld,hbd->hbl', k, q)  masked softmax over l
  out = LeakyReLU_0.4(einsum('hbl,blf->bhf', attn, bank))

Strategy: data-parallel over batch B across 8 NeuronCores (8 b's per core).

Sparsity: the mask zeroes ~half the L positions; masked positions receive
-1e8 bias so their softmax weight is ~0 and they contribute nothing to the
output.  Host prep therefore COMPACTS bank per-b to the unmasked columns
(padded to Lp, a multiple of 32; padded slots keep the -1e8 bias), which
cuts the dominant k-matmul, the score matmul and the softmax by L/Lp
(~1.6x).  Compaction is a gather (re-layout); all FLOPs stay on device.

Device pipeline per core (PE stream is issued to stay gap-free):
  - k = tanh(KeyT^T @ bankT) head-outer (KeyT streams one head per ~5us of
    PE work), all 4 b-pairs; moving dim 2*Lp>=256 so fp32r runs full rate.
  - q-heads (bf16, halves the Query DMA) interleaved into the first k-heads
    to cover the DMA lead-in; q is DVE-block-transposed into the zero-padded
    score lhsT (qz) -- no PE transposes anywhere.
  - scores: all (h,dc) accumulate into one [40, 2*Lp] psum per b-pair;
    masked softmax on ACT/DVE; attn DVE-block-transposed; emb = attn @ bank
    with normalize+LeakyReLU fused into one Prelu activation.
"""

import os
import numpy as np
import concourse.bass as bass  # noqa: F401
import concourse.mybir as mybir
import concourse.tile as tile
from concourse import bacc, bass_utils

F32 = mybir.dt.float32
F32R = mybir.dt.float32r
FP16 = mybir.dt.float16
AF = mybir.ActivationFunctionType
AX = mybir.AxisListType

# dtype of the big k-matmul operands (KeyT / bankT). fp16 halves their DMA
# at ~0.05% quantization error; fp32r keeps tf32-grade accuracy.
KF16 = os.environ.get("KERNEL_KF16", "1") == "1"

H, D, E, F = 8, 256, 1536, 768
B, L = 64, 256
NCORES = 8
BPC = B // NCORES          # 8 b's per core
NBP = BPC // 2             # 4 b-pairs per core
EC, FC, DC = E // 128, F // 128, D // 128   # 12, 6, 2


def _build_program(Lp, kf16):
    KMM = FP16 if kf16 else F32R
    N2 = 2 * Lp                 # k / score moving width per b-pair
    Lpt = -(-Lp // 32) * 32     # Lp padded to the DVE 32x32 transpose grid
    NBLK = Lpt // 32            # transpose blocks per attn stack
    L_REM = Lp - 128 if Lp > 128 else 0   # l rows beyond the first 128

    nc = bacc.Bacc("TRN2", target_bir_lowering=False, debug=False,
                   enable_asserts=False, num_devices=NCORES)
    qt = nc.dram_tensor("qt", [H, E, D], FP16, kind="ExternalInput").ap()
    kt = nc.dram_tensor("kt", [H, F, D], KMM, kind="ExternalInput").ap()
    bkt = nc.dram_tensor("bkt", [NBP, F, N2], KMM, kind="ExternalInput").ap()
    bkn = nc.dram_tensor("bkn", [BPC, Lp, F], FP16, kind="ExternalInput").ap()
    xt = nc.dram_tensor("xt", [128, EC * BPC], FP16, kind="ExternalInput").ap()
    mb = nc.dram_tensor("mb", [BPC, H, Lp], F32, kind="ExternalInput").ap()
    out = nc.dram_tensor("out", [BPC, H, F], F32, kind="ExternalOutput").ap()

    with tile.TileContext(nc) as tc:
        with tc.tile_pool(name="const", bufs=1) as cpool, \
             tc.tile_pool(name="weights", bufs=1) as wpool, \
             tc.tile_pool(name="stream", bufs=4) as spool, \
             tc.tile_pool(name="small", bufs=2) as smpool, \
             tc.tile_pool(name="psK", bufs=3, space="PSUM") as psK, \
             tc.tile_pool(name="psS", bufs=2, space="PSUM") as psS, \
             tc.tile_pool(name="psM", bufs=3, space="PSUM") as psM:

            # ---------------- resident SBUF tiles ------------------------
            kt_tiles = [wpool.tile([128, FC * D], KMM, name=f"kt_sb{h}",
                                   tag=f"kt_sb{h}") for h in range(H)]
            # bankT, all b-pairs resident: [128(f), bp, fc, (b2 l)]
            bktA = cpool.tile([128, NBP * FC * N2], KMM)
            bktA_v = bktA[:].rearrange("p (bp fc n) -> p bp fc n", bp=NBP, fc=FC)
            # bank (natural layout), emb rhs: first 128 l-rows + remainder.
            bkn0 = cpool.tile([128, BPC * F], FP16, name="bkn0")
            bkn1 = (cpool.tile([L_REM, BPC * F], FP16, name="bkn1")
                    if L_REM else None)
            xt_sb = cpool.tile([128, EC * BPC], FP16)
            mbA = cpool.tile([8, BPC * Lp], F32)
            # zero-padded score lhsT: col = bp*640 + (2h+dc)*40 + 32*b2 + h
            qz = cpool.tile([128, NBP * 640], FP16)
            qz_v = qz[:].rearrange("p (bp blk c) -> p bp blk c", bp=NBP, blk=16)
            # k = tanh(...), all heads/pairs resident: [128(d), h, dc, bp, n2]
            k_t = cpool.tile([128, H * DC * NBP * N2], FP16)
            k_v = k_t[:].rearrange("p (h dc bp n) -> p h dc bp n",
                                   h=H, dc=DC, bp=NBP)

            # ---------------- DMA issue helpers --------------------------
            def load_xt_mb():
                nc.sync.dma_start(xt_sb[:], xt)
                nc.sync.dma_start(
                    mbA[:].rearrange("h (b l) -> h b l", b=BPC),
                    mb.rearrange("b h l -> h b l"))

            def load_kt(h, pieces=2):
                fc_per = FC // pieces
                for piece in range(pieces):
                    nc.sync.dma_start(
                        kt_tiles[h][:, piece * fc_per * D:(piece + 1) * fc_per * D]
                        .rearrange("p (fc d) -> p fc d", fc=fc_per),
                        kt[h, piece * fc_per * 128:(piece + 1) * fc_per * 128]
                        .rearrange("(fc p) d -> p fc d", p=128))

            def load_kt_piece(h, piece, pieces):
                fc_per = FC // pieces
                nc.sync.dma_start(
                    kt_tiles[h][:, piece * fc_per * D:(piece + 1) * fc_per * D]
                    .rearrange("p (fc d) -> p fc d", fc=fc_per),
                    kt[h, piece * fc_per * 128:(piece + 1) * fc_per * 128]
                    .rearrange("(fc p) d -> p fc d", p=128))

            def load_bktA_piece(bp, piece, pieces):
                fc_per = FC // pieces
                nc.sync.dma_start(
                    bktA_v[:, bp, piece * fc_per:(piece + 1) * fc_per],
                    bkt[bp, piece * fc_per * 128:(piece + 1) * fc_per * 128]
                    .rearrange("(fc p) n -> p fc n", p=128))

            def load_bktA(bp, pieces=2):
                fc_per = FC // pieces
                for piece in range(pieces):
                    nc.sync.dma_start(
                        bktA_v[:, bp, piece * fc_per:(piece + 1) * fc_per],
                        bkt[bp, piece * fc_per * 128:(piece + 1) * fc_per * 128]
                        .rearrange("(fc p) n -> p fc n", p=128))

            def load_bkn():
                for b in range(BPC):
                    nc.sync.dma_start(
                        bkn0[:, b * F:(b + 1) * F], bkn[b, 0:128])
                    if L_REM:
                        nc.sync.dma_start(
                            bkn1[:, b * F:(b + 1) * F], bkn[b, 128:128 + L_REM])

            # ---------------- q phase (bf16) ------------------------------
            # q = tanh(x @ Query^T); two stacks of 4 heads (rows 32*hh, 8
            # live rows each -- engine writes need 32-aligned partition
            # bases) -> DVE 32x32 block transposes -> strided copies into qz.
            q_stacks = [smpool.tile([128, D], FP16, name=f"qs{g}", tag=f"qs{g}")
                        for g in range(2)]

            def q_head(h):
                g, hh = divmod(h, 4)
                pq = psM.tile([BPC, D], F32, name="pq", tag="psm")
                qt_c = spool.tile([128, EC * D], FP16, name="qt_c", tag="qt_c")
                nc.sync.dma_start(
                    qt_c[:].rearrange("p (ec d) -> p ec d", ec=EC),
                    qt[h].rearrange("(ec p) d -> p ec d", p=128))
                for ec in range(EC):
                    nc.tensor.matmul(pq[:], xt_sb[:, ec * BPC:(ec + 1) * BPC],
                                     qt_c[:, ec * D:(ec + 1) * D],
                                     start=(ec == 0), stop=(ec == EC - 1))
                nc.scalar.activation(q_stacks[g][32 * hh:32 * hh + 8, :], pq[:],
                                     AF.Tanh)

            def q_scatter(g):
                for dc in range(DC):
                    qT = smpool.tile([128, 128], FP16, name=f"qT{g}{dc}",
                                     tag="qT")
                    for i in range(4):          # head row-blocks
                        for j in range(4):      # d sub-blocks
                            nc.vector.transpose(
                                qT[32 * j:32 * j + 32, 32 * i:32 * i + 32],
                                q_stacks[g][32 * i:32 * i + 32,
                                            dc * 128 + 32 * j:dc * 128 + 32 * j + 32])
                    # qT col = 32*hh + 2*bp + b2 (b = 2bp+b2 local batch)
                    qT_v = qT[:].rearrange("p (hh bpx b2) -> p hh bpx b2",
                                           hh=4, bpx=16)
                    for hh in range(4):
                        h = 4 * g + hh
                        for b2 in range(2):
                            nc.vector.tensor_copy(
                                qz_v[:, :, 2 * h + dc, 32 * b2 + h],
                                qT_v[:, hh, 0:4, b2])

            # ---------------- k phase -------------------------------------
            def k_head(h, bps=range(NBP)):
                for bp in bps:
                    for dc in range(DC):
                        pk = psK.tile([128, N2], F32, name="pk", tag="pk")
                        for fc in range(FC):
                            nc.tensor.matmul(
                                pk[:],
                                kt_tiles[h][:, fc * D + dc * 128:
                                            fc * D + dc * 128 + 128],
                                bktA_v[:, bp, fc],
                                start=(fc == 0), stop=(fc == FC - 1))
                        nc.scalar.activation(k_v[:, h, dc, bp], pk[:], AF.Tanh)

            # ---------------- score / softmax / emb -----------------------
            simsafe = os.environ.get("KERNEL_SIM_SAFE", "0") == "1"

            def score_mms(bp):
                ps40 = psS.tile([40, N2], F32, name="ps40", tag="ps40")
                for h in range(H):
                    for dc in range(DC):
                        nc.tensor.matmul(
                            ps40[:], qz_v[:, bp, 2 * h + dc],
                            k_v[:, h, dc, bp],
                            start=(h == 0 and dc == 0),
                            stop=(h == H - 1 and dc == DC - 1))
                return ps40

            def softmax(bp, ps40):
                # both b2 stacked at rows 0 / 32: one exp, one reciprocal,
                # one Prelu scale vector for the whole b-pair.
                s40 = smpool.tile([40, Lp], F32, name="s40", tag="s40")
                nm40 = smpool.tile([40, 1], F32, name="nm40", tag="nm40")
                zs40 = smpool.tile([40, 1], F32, name="zs40", tag="zs40")
                rz40 = smpool.tile([40, 1], F32, name="rz40", tag="rz40")
                p40 = smpool.tile([64, Lpt], FP16, name="p40", tag="p40")
                for b2 in range(2):
                    b = 2 * bp + b2
                    nc.vector.tensor_add(s40[32 * b2:32 * b2 + 8, :],
                                         ps40[32 * b2:32 * b2 + 8,
                                              Lp * b2:Lp * b2 + Lp],
                                         mbA[:, b * Lp:(b + 1) * Lp])
                    nc.vector.reduce_max(nm40[32 * b2:32 * b2 + 8, :],
                                         s40[32 * b2:32 * b2 + 8, :],
                                         axis=AX.X, negate=True)
                nc.scalar.activation(p40[0:40, 0:Lp], s40[:], AF.Exp,
                                     bias=nm40[:], accum_out=zs40[:])
                nc.vector.reciprocal(rz40[:], zs40[:])
                pT0 = smpool.tile([128, 64], FP16, name="pT0", tag="pT0")
                pT1 = (smpool.tile([max(L_REM, 32), 64], FP16,
                                   name="pT1", tag="pT1")
                       if L_REM else None)
                for i in range(2):              # b2 row-blocks
                    for j in range(NBLK):
                        row = 32 * j
                        dst = (pT0[row:row + 32, 32 * i:32 * i + 32]
                               if row < 128
                               else pT1[row - 128:row - 96, 32 * i:32 * i + 32])
                        nc.vector.transpose(
                            dst, p40[32 * i:32 * i + 32, row:row + 32])
                return rz40, pT0, pT1

            def emb(bp, rz40, pT0, pT1):
                o40 = smpool.tile([40, F], F32, name="o40", tag="o40")
                for fh in range(2):
                    pe = psM.tile([40, 384], F32, name="pe", tag="psm")
                    for b2 in range(2):
                        b = 2 * bp + b2
                        nc.tensor.matmul(
                            pe[32 * b2:32 * b2 + 8, :],
                            pT0[:, 32 * b2:32 * b2 + 8],
                            bkn0[:, b * F + fh * 384:b * F + fh * 384 + 384],
                            start=True, stop=(L_REM == 0))
                        if L_REM:
                            nc.tensor.matmul(
                                pe[32 * b2:32 * b2 + 8, :],
                                pT1[0:L_REM, 32 * b2:32 * b2 + 8],
                                bkn1[:, b * F + fh * 384:b * F + fh * 384 + 384],
                                start=False, stop=True)
                    if simsafe:
                        nc.scalar.activation(o40[:, fh * 384:fh * 384 + 384],
                                             pe[0:40, :], AF.Copy,
                                             scale=rz40[:])
                    else:
                        nc.scalar.activation(o40[:, fh * 384:fh * 384 + 384],
                                             pe[0:40, :], AF.Prelu,
                                             scale=rz40[:], alpha=0.4)
                for b2 in range(2):
                    nc.sync.dma_start(out[2 * bp + b2],
                                      o40[32 * b2:32 * b2 + 8, :])

            # ---------------- program order -------------------------------
            # Front-load the high-leverage DMAs: each kt[h] (1.1us of DMA)
            # unlocks 5.1us of PE work, bankT is shared by all heads.  The
            # Query stream (1:1 DMA:PE) queues behind them and is consumed
            # mid-sweep when everything is already resident.
            nc.vector.memset(qz[:], 0.0)
            for piece in range(3):
                load_kt_piece(0, piece, 3)
                load_bktA_piece(0, piece, 3)
            nc.sync.dma_start(xt_sb[:], xt)
            load_bktA(1, pieces=1)
            load_bktA(2, pieces=1)
            load_bktA(3, pieces=1)
            load_kt(1, pieces=1)
            load_kt(2, pieces=1)
            load_kt(3, pieces=1)
            k_head(0)
            k_head(1)
            k_head(2)
            for h in range(4):
                q_head(h)
            q_scatter(0)
            k_head(3)
            for h in range(4, H):
                q_head(h)
            q_scatter(1)
            load_kt(4, pieces=1)
            k_head(4)
            load_kt(5, pieces=1)
            nc.sync.dma_start(
                mbA[:].rearrange("h (b l) -> h b l", b=BPC),
                mb.rearrange("b h l -> h b l"))
            k_head(5)
            load_kt(6, pieces=1)
            load_bkn()
            k_head(6)
            load_kt(7, pieces=1)
            k_head(7)

            # all scores, then all embs: the last softmax chain overlaps
            # the first embs, so the PE never waits on it.
            ps0 = score_mms(0)
            sm0 = softmax(0, ps0)
            ps1 = score_mms(1)
            sm1 = softmax(1, ps1)
            ps2 = score_mms(2)
            sm2 = softmax(2, ps2)
            ps3 = score_mms(3)
            sm3 = softmax(3, ps3)
            emb(0, *sm0)
            emb(1, *sm1)
            emb(2, *sm2)
            emb(3, *sm3)

    nc.finalize()
    return nc


def _host_prep(x, bank, mask, Query, Key, Lp, kf16):
    x = np.ascontiguousarray(x, dtype=np.float32)
    bank = np.ascontiguousarray(bank, dtype=np.float32)
    Query = np.ascontiguousarray(Query, dtype=np.float32)
    Key = np.ascontiguousarray(Key, dtype=np.float32)
    mask = np.asarray(mask)

    kdt = np.float16 if kf16 else np.float32
    qt = np.ascontiguousarray(Query.transpose(0, 2, 1)).astype(np.float16)
    kt = np.ascontiguousarray(Key.transpose(0, 2, 1)).astype(kdt)  # [H, F, D]

    # per-b compaction of bank to its unmasked columns, padded to Lp
    if Lp == L:
        bank_c = bank
        mbias = np.where(mask == 0, np.float32(-1e8), np.float32(0.0))
    else:
        idx = np.zeros((B, Lp), dtype=np.int64)
        mbias = np.full((B, Lp), np.float32(-1e8))
        for b in range(B):
            nz = np.flatnonzero(mask[b])
            idx[b, :len(nz)] = nz
            mbias[b, :len(nz)] = 0.0
        bank_c = np.take_along_axis(bank, idx[:, :, None], axis=1)
    mbias = mbias.astype(np.float32)

    in_maps = []
    for c in range(NCORES):
        bs = c * BPC
        bc = bank_c[bs:bs + BPC]                      # [BPC, Lp, F]
        # bkt: [NBP, F, 2*Lp] -- b-pair minor so one DMA pair per bp
        bkt_c = np.ascontiguousarray(
            bc.reshape(NBP, 2, Lp, F).transpose(0, 3, 1, 2)
            .reshape(NBP, F, 2 * Lp)).astype(kdt)
        xs = x[bs:bs + BPC]                           # [BPC, E]
        xt_c = np.ascontiguousarray(
            xs.T.reshape(EC, 128, BPC).transpose(1, 0, 2)
            .reshape(128, EC * BPC)).astype(np.float16)
        mb_c = np.ascontiguousarray(
            np.repeat(mbias[bs:bs + BPC, None, :], H, axis=1))
        in_maps.append({
            "qt": qt,
            "kt": kt,
            "bkt": bkt_c,
            "bkn": np.ascontiguousarray(bc).astype(np.float16),
            "xt": xt_c,
            "mb": mb_c,
        })
    return in_maps


_NC_CACHE = {}


def _pick_lp(mask):
    counts = np.asarray(mask).astype(bool).sum(axis=1)
    if counts.min() == 0:
        return L
    return int(min(L, max(128, -(-int(counts.max()) // 8) * 8)))


def kernel(x, bank, mask, Query, Key):
    Lp = _pick_lp(mask)
    key = (Lp, KF16)
    if key not in _NC_CACHE:
        _NC_CACHE[key] = _build_program(Lp, KF16)
    nc = _NC_CACHE[key]
    in_maps = _host_prep(x, bank, mask, Query, Key, Lp, KF16)

    trace = os.environ.get("KERNEL_TRACE", "0") == "1"
    res = bass_utils.run_bass_kernel_spmd(nc, in_maps,
                                          core_ids=list(range(NCORES)),
                                          trace=trace)
    if trace:
        print("exec_time_ns:", res.exec_time_ns,
              "mean:", res.mean_exec_time_ns,
              "core:", res.max_exec_time_core_id)
    return np.concatenate([r["out"] for r in res.results], axis=0)
